# revision 35
# baseline (speedup 1.0000x reference)
"""Multi-head causal attention (B=2, T=4096, C=768, H=12) on 8 TRN2 NeuronCores.

Sharding: 24 (batch, head) units -> 3 heads per core; cores 0-3 take batch 0,
cores 4-7 batch 1. Each core computes Q/K/V projections for its 3 heads, full-T
causal attention, and a partial output projection [C, T]. Host sums the 4
partials per batch and adds the output bias (+ Wo @ bv, folded out of the
device kernel).

Pipeline structure (per q-tile qi): the Q/K/V projections for tile qi+1 and
the out-projection for tile qi-1 are emitted as filler units spread through
qi's attention k-loop, so the PE never idles while the ACT engine (exp) is
the steady-state bottleneck, and the PE clock stays warm (HAM).

Device layouts (per core):
  xT    [768, T] bf16   x[b] transposed (c-major) - input
  Q, K  [d, t] bf16     head-pair tiles [128, 512] (partitions = 2x64 head dims)
  q2d, k2t [128, 512]   head-2 Q/K duplicated on both partition halves so
                        h2's QK matmuls for even/odd k-subtiles pair onto
                        disjoint PE row groups
  V     [t, d] bf16     per 128-row tile [128, 3*65] (65th col = ones -> denom)
  att^T [k, q]          QK^T computed transposed; heads 0/1 on PE row groups
                        0-1 / 2-3 run concurrently; h2 even/odd kt likewise
  exp   bf16            ACT Exp from PSUM; diagonal tiles column-trimmed;
                        causal mask applied as 0/1 multiply on the staircase
  y     [65, 512] psum  accum over k-tiles (row 64 = softmax denominator)
  yt01  [128, 512] bf16 normalized y for heads 0+1 stacked (K=128 out-proj)
  out   [768, T] f32    partial out-projection, c_out-major
"""

import ml_dtypes
import numpy as np

import concourse.bass as bass
import concourse.tile as tile
from concourse import bacc, mybir

F32 = mybir.dt.float32
BF16 = mybir.dt.bfloat16
AF = mybir.ActivationFunctionType

N_CORES = 8
T = 4096
C = 768
H = 12
D = 64
HPC = 3          # heads per core
QT = 512         # q-tile width (matmul N)
KT = 128         # k-tile width (partition dim)
NCH = C // 128   # 6 contraction chunks over C


def build_nc(t=T):
    nt = t // QT          # q/t tiles of 512
    nsub = t // KT        # t sub-tiles of 128

    nc = bacc.Bacc("TRN2", target_bir_lowering=False, debug=False)

    xT = nc.declare_dram_parameter("xT", [C, t], BF16, isOutput=False)
    wqk = nc.declare_dram_parameter("wqk", [C, 384], BF16, isOutput=False)
    bqk = nc.declare_dram_parameter("bqk", [128, 3], F32, isOutput=False)
    wv = nc.declare_dram_parameter("wv", [C, 192], BF16, isOutput=False)
    wo01 = nc.declare_dram_parameter("wo01", [128, C], BF16, isOutput=False)
    wo2 = nc.declare_dram_parameter("wo2", [64, C], BF16, isOutput=False)
    msk = nc.declare_dram_parameter("msk", [128, 4 * QT], BF16, isOutput=False)
    out = nc.declare_dram_parameter("out", [C, t], F32, isOutput=True)

    xT_r = xT.ap().rearrange("(a p) t -> p a t", p=128)
    wqk_r = wqk.ap().rearrange("(a p) m -> p a m", p=128)
    wv_r = wv.ap().rearrange("(a p) m -> p a m", p=128)

    with tile.TileContext(nc) as tc:
        with (
            tc.tile_pool(name="const", bufs=1) as const_pool,
            tc.tile_pool(name="xt", bufs=4 * NCH) as xp,
            tc.tile_pool(name="qp", bufs=nt) as qp,
            tc.tile_pool(name="kp", bufs=nt) as kp,
            tc.tile_pool(name="q2p", bufs=nt) as q2p,
            tc.tile_pool(name="k2p", bufs=nt) as k2p,
            tc.tile_pool(name="vp", bufs=nsub) as vp,
            tc.tile_pool(name="yp", bufs=nt) as yp,
            tc.tile_pool(name="ep", bufs=16) as ep,
            tc.tile_pool(name="op", bufs=8) as op,
            tc.tile_pool(name="sp", bufs=6) as sp,
            tc.tile_pool(name="ps_sc", bufs=2, space="PSUM") as ps_sc,
            tc.tile_pool(name="ps_b", bufs=4, space="PSUM") as ps_b,
        ):
            # ---- first x tile + hot constants first (head latency) ----
            xtiles = {}

            def emit_xt_dma(ti):
                xtc = []
                for ci in range(NCH):
                    xc = xp.tile([128, QT], BF16, tag="xt")
                    nc.sync.dma_start(
                        out=xc, in_=xT_r[:, ci, ti * QT:(ti + 1) * QT]
                    )
                    xtc.append(xc)
                xtiles[ti] = xtc

            wqk_sb = const_pool.tile([128, NCH, 384], BF16, tag="wqk")
            # m0 weights + x chunks first: first proj matmul gates the head
            nc.sync.dma_start(out=wqk_sb[:, :, 0:128], in_=wqk_r[:, :, 0:128])
            emit_xt_dma(0)
            for mm_ in (1, 2):
                nc.sync.dma_start(
                    out=wqk_sb[:, :, mm_ * 128:(mm_ + 1) * 128],
                    in_=wqk_r[:, :, mm_ * 128:(mm_ + 1) * 128],
                )
            bqk_sb = const_pool.tile([128, 3], F32, tag="bqk")
            nc.sync.dma_start(out=bqk_sb, in_=bqk.ap())
            wv_sb = const_pool.tile([128, NCH, 192], BF16, tag="wv")
            nc.sync.dma_start(out=wv_sb, in_=wv_r)
            mask_sb = const_pool.tile([128, 4, QT], BF16, tag="msk")
            msk_r = msk.ap().rearrange("p (o q) -> p o q", q=QT)
            nc.sync.dma_start(out=mask_sb, in_=msk_r)
            wo01_sb = const_pool.tile([128, C], BF16, tag="wo01")
            nc.sync.dma_start(out=wo01_sb, in_=wo01.ap())
            wo2_sb = const_pool.tile([64, C], BF16, tag="wo2")
            nc.sync.dma_start(out=wo2_sb, in_=wo2.ap())

            # HAM warm-up: ~5us of dependency-free dummy matmuls keep the
            # PE busy while the head DMAs stream, so real work starts at the
            # full 2.4 GHz clock instead of the cold 1.2 GHz half-rate.
            warm_in = const_pool.tile([128, QT], BF16, tag="warm")
            nc.vector.memset(warm_in, 0.0)
            warm_ps = ps_sc.tile([128, 2, QT], F32, tag="ps", name="warm_ps")
            for _ in range(24):
                nc.tensor.matmul(
                    warm_ps[:, 0, :], lhsT=warm_in[:, 0:128], rhs=warm_in,
                    start=True, stop=True,
                )

            q_t = [None] * nt
            k_t = [None] * nt
            q2_t = [None] * nt
            k2_t = [None] * nt
            v_t = [None] * (4 * nt)
            yt01_t = [None] * nt
            yt2_t = [None] * nt

            # ---------- emission units ----------

            def proj_m01(ti, m):
                # m = 0: Q heads 0/1; m = 1: K heads 0/1. M = 128.
                ps = ps_b.tile([128, QT], F32, tag="psb")
                for ci in range(NCH):
                    nc.tensor.matmul(
                        ps,
                        lhsT=wqk_sb[:, ci, m * 128:(m + 1) * 128],
                        rhs=xtiles[ti][ci],
                        start=(ci == 0),
                        stop=(ci == NCH - 1),
                    )
                pool = (qp, kp)[m]
                dst = pool.tile([128, QT], BF16, tag=("q", "k")[m])
                nc.vector.tensor_scalar_add(dst, ps, bqk_sb[:, m:m + 1])
                if m == 0:
                    q_t[ti] = dst
                else:
                    k_t[ti] = dst

            def proj_m2(ti):
                # [Q2; K2] stacked: q2 on partitions 0-63, k2 on 64-127.
                ps = ps_b.tile([128, QT], F32, tag="psb")
                for ci in range(NCH):
                    nc.tensor.matmul(
                        ps,
                        lhsT=wqk_sb[:, ci, 256:384],
                        rhs=xtiles[ti][ci],
                        start=(ci == 0),
                        stop=(ci == NCH - 1),
                    )
                q2d = q2p.tile([128, QT], BF16, tag="q2")
                k2t = k2p.tile([128, QT], BF16, tag="k2")
                nc.vector.tensor_scalar_add(
                    q2d[0:64, :], ps[0:64, :], bqk_sb[0:64, 2:3]
                )
                nc.vector.tensor_scalar_add(
                    k2t[64:128, :], ps[64:128, :], bqk_sb[64:128, 2:3]
                )
                # Duplicate to the other partition half (DMA partition move,
                # off the PE critical path) so h2's QK matmuls pair onto
                # disjoint PE row groups.
                nc.sync.dma_start(out=q2d[64:128, :], in_=q2d[0:64, :])
                nc.sync.dma_start(out=k2t[0:64, :], in_=k2t[64:128, :])
                q2_t[ti] = q2d
                k2_t[ti] = k2t

            def proj_v(ti, si):
                psv = ps_b.tile([128, 192], F32, tag="psb")
                for ci in range(NCH):
                    nc.tensor.matmul(
                        psv,
                        lhsT=xtiles[ti][ci][:, si * 128:(si + 1) * 128],
                        rhs=wv_sb[:, ci, :],
                        start=(ci == 0),
                        stop=(ci == NCH - 1),
                    )
                vt = vp.tile([128, HPC * 65], BF16, tag="v")
                vt_r = vt.rearrange("p (h e) -> p h e", e=65)
                nc.vector.memset(vt_r[:, :, 64:65], 1.0)
                nc.vector.tensor_copy(
                    vt_r[:, :, 0:64],
                    psv[:, 0:HPC * 64].rearrange("p (h e) -> p h e", e=64),
                )
                v_t[4 * ti + si] = vt

            def proj_units(ti):
                yield lambda: proj_m01(ti, 0)
                yield lambda: proj_m01(ti, 1)
                yield lambda: proj_m2(ti)
                for si in range(4):
                    yield lambda si=si: proj_v(ti, si)

            def outproj_mo(qi, mo, wo2_first=False):
                # For the last q-tile the wo2 matmul goes first: yt2 is
                # normalized first there, so the out-projection overlaps the
                # rest of the normalize chain instead of waiting for yt01.
                ps = ps_b.tile([128, QT], F32, tag="psb")
                pair = [
                    (wo01_sb, yt01_t[qi]),
                    (wo2_sb, yt2_t[qi]),
                ]
                if wo2_first:
                    pair.reverse()
                for i, (w, y) in enumerate(pair):
                    nc.tensor.matmul(
                        ps,
                        lhsT=w[:, mo * 128:(mo + 1) * 128],
                        rhs=y,
                        start=(i == 0),
                        stop=(i == 1),
                    )
                ot = op.tile([128, QT], F32, tag="o")
                nc.vector.tensor_copy(ot, ps)
                nc.sync.dma_start(
                    out=out.ap()[mo * 128:(mo + 1) * 128,
                                 qi * QT:(qi + 1) * QT],
                    in_=ot,
                )

            def outproj_units(qi):
                last = qi == nt - 1
                for mo in range(NCH):
                    yield lambda mo=mo: outproj_mo(qi, mo, wo2_first=last)

            def normalize(qi, h, y_ps, dst_ap):
                # y_ps row 64 = denominator. DVE-copy it straight to SBUF
                # partition 0 (64->0 is a quadrant-aligned cross-quadrant
                # move), reciprocal on 1 partition, broadcast on GpSimd,
                # multiply on DVE.
                den = sp.tile([1, QT], F32, tag="den")
                nc.vector.tensor_copy(den, y_ps[64:65, :])
                rec = sp.tile([1, QT], F32, tag="rec")
                nc.vector.reciprocal_approx_fast(rec, den)
                bc = sp.tile([64, QT], F32, tag="bcs")
                nc.gpsimd.partition_broadcast(bc, rec[0:1, :])
                nc.vector.tensor_mul(dst_ap, y_ps[0:64, :], bc)

            # ---------- main pipeline ----------
            filler = []          # queue of pending PE filler units

            def emit_fillers(k):
                for _ in range(k):
                    if filler:
                        filler.pop(0)()

            # prologue: only Q01/K01 of tile 0 up front -- att(0)'s QK/exp
            # can then start immediately; m2 + V projections arrive as the
            # first fillers, just in time for h2's QK and the deferred attV.
            proj_m01(0, 0)
            proj_m01(0, 1)
            filler.append(lambda: proj_m2(0))
            for si in range(4):
                filler.append(lambda si=si: proj_v(0, si))
            if nt > 1:
                emit_xt_dma(1)
                filler.extend(proj_units(1))

            for qi in range(nt):
                n_k = 4 * qi + 4
                q0_ap = q_t[qi][0:64, :]
                q1_ap = q_t[qi][64:128, :]
                y0 = ps_b.tile([128, QT], F32, tag="psb")
                y1 = ps_b.tile([128, QT], F32, tag="psb")
                y2 = ps_b.tile([128, QT], F32, tag="psb")
                # filler spacing: spread pending units across this kt loop;
                # front-load a chunk so the PE has work queued ahead of the
                # first attV flushes while normalize(qi-1) releases y slots
                n_fill = len(filler)
                done_fill = 0
                emit_fillers(max(0, n_fill // 4))
                done_fill = min(n_fill, max(0, n_fill // 4))

                # attV is deferred by one k-tile (h01) / one pair (h2) so the
                # exp it consumes finished during the previous iteration's PE
                # work -- avoids a PE pipeline restart per k-tile.
                pend01 = None   # (kt, et, lo)
                pend2 = None    # (kt0, et2)

                def flush01():
                    nonlocal pend01
                    if pend01 is None:
                        return
                    pkt, pet, plo = pend01
                    nc.tensor.matmul(
                        y0[0:65, plo:QT],
                        lhsT=v_t[pkt][:, 0:65],
                        rhs=pet[:, 0, plo:QT],
                        start=(pkt == 0),
                        stop=(pkt == n_k - 1),
                    )
                    nc.tensor.matmul(
                        y1[0:65, plo:QT],
                        lhsT=v_t[pkt][:, 65:130],
                        rhs=pet[:, 1, plo:QT],
                        start=(pkt == 0),
                        stop=(pkt == n_k - 1),
                    )
                    pend01 = None

                def flush2():
                    nonlocal pend2
                    if pend2 is None:
                        return
                    pkt0, pet2 = pend2
                    for u in (0, 1):
                        ku = pkt0 + u
                        lou = max(0, ku - 4 * qi) * 128
                        nc.tensor.matmul(
                            y2[0:65, lou:QT],
                            lhsT=v_t[ku][:, 130:195],
                            rhs=pet2[:, u, lou:QT],
                            start=(ku == 0),
                            stop=(ku == n_k - 1),
                        )
                    pend2 = None

                for kt in range(n_k):
                    # fillers first: they absorb the wait for the score-slot
                    # release (ACT-bound cadence) instead of a PE restart
                    want = ((kt + 1) * n_fill) // n_k
                    emit_fillers(want - done_fill)
                    done_fill = max(done_fill, want)

                    tj, tcol = kt // 4, (kt % 4) * 128
                    o = kt - 4 * qi
                    lo = max(0, o) * 128   # first unmasked column
                    aps = ps_sc.tile([128, 2, QT], F32, tag="ps")
                    nc.tensor.matmul(
                        aps[:, 0, lo:QT], lhsT=k_t[tj][0:64, tcol:tcol + 128],
                        rhs=q0_ap[:, lo:QT], start=True, stop=True,
                    )
                    nc.tensor.matmul(
                        aps[:, 1, lo:QT], lhsT=k_t[tj][64:128, tcol:tcol + 128],
                        rhs=q1_ap[:, lo:QT], start=True, stop=True,
                    )
                    et = ep.tile([128, 2, QT], BF16, tag="e")
                    nc.scalar.activation(et[:, :, lo:QT], aps[:, :, lo:QT], AF.Exp)
                    if o >= 0:
                        nc.vector.tensor_mul(
                            et[:, :, lo:QT], et[:, :, lo:QT],
                            mask_sb[:, o:o + 1, lo:QT].to_broadcast(
                                [128, 2, QT - lo]
                            ),
                        )
                    flush01()
                    pend01 = (kt, et, lo)

                    # ---- head 2: one pair of k-tiles per exp ----
                    if kt % 2 == 1:
                        kt0 = kt - 1
                        o0 = max(0, kt0 - 4 * qi)
                        lo0 = o0 * 128
                        aps2 = ps_sc.tile([128, 2, QT], F32, tag="ps")
                        for u in (0, 1):
                            ku = kt0 + u
                            tju, tcu = ku // 4, (ku % 4) * 128
                            base = 64 * (ku % 2)
                            # both slots cover [lo0:QT] so exp never reads
                            # never-written PSUM; mask zeroes the staircase
                            nc.tensor.matmul(
                                aps2[:, u, lo0:QT],
                                lhsT=k2_t[tju][base:base + 64, tcu:tcu + 128],
                                rhs=q2_t[qi][base:base + 64, lo0:QT],
                                start=True, stop=True,
                            )
                        et2 = ep.tile([128, 2, QT], BF16, tag="e")
                        nc.scalar.activation(
                            et2[:, :, lo0:QT], aps2[:, :, lo0:QT], AF.Exp
                        )
                        for u in (0, 1):
                            ou = kt0 + u - 4 * qi
                            if ou >= 0:
                                nc.vector.tensor_mul(
                                    et2[:, u, lo0:QT], et2[:, u, lo0:QT],
                                    mask_sb[:, ou, lo0:QT],
                                )
                        flush2()
                        pend2 = (kt0, et2)

                flush01()
                flush2()

                # ---- normalize ----
                yt01 = yp.tile([128, QT], BF16, tag="y01")
                yt2 = yp.tile([64, QT], BF16, tag="y2")
                if qi == nt - 1:
                    normalize(qi, 2, y2, yt2)
                normalize(qi, 0, y0, yt01[0:64, :])
                # h1 writes cross-quadrant (banks 0,1 -> quadrants 2,3)
                normalize(qi, 1, y1, yt01[64:128, :])
                if qi != nt - 1:
                    normalize(qi, 2, y2, yt2)
                yt01_t[qi] = yt01
                yt2_t[qi] = yt2

                emit_fillers(len(filler))

                # refill the filler queue for the next iteration
                if qi + 2 < nt:
                    emit_xt_dma(qi + 2)
                    filler.extend(proj_units(qi + 2))
                filler.extend(outproj_units(qi))

            # drain remaining fillers (last outproj)
            emit_fillers(len(filler))

    nc.compile()
    return nc


def make_mask():
    i = np.arange(128)[:, None]
    j = np.arange(QT)[None, :]
    m = np.zeros((128, 4 * QT), np.float32)
    for o in range(4):
        m[:, o * QT:(o + 1) * QT] = (j >= o * 128 + i)
    return m


def shard_inputs(x, Wq, bq, Wk, bk, Wv, bv, Wo, bo, t=T):
    """Build per-core in_maps."""
    s = 1.0 / np.sqrt(D)
    mask = make_mask()
    bf = ml_dtypes.bfloat16
    in_maps = []
    for c in range(N_CORES):
        b = c // (N_CORES // x.shape[0])
        h0 = HPC * (c % 4)
        hs = slice(h0 * D, (h0 + HPC) * D)
        Wq_s = (Wq[hs] * s).astype(np.float32)
        bq_s = (bq[hs] * s).astype(np.float32)
        Wk_s, bk_s = Wk[hs], bk[hs]
        wqk = np.concatenate(
            [Wq_s[0:128].T, Wk_s[0:128].T, Wq_s[128:192].T, Wk_s[128:192].T],
            axis=1,
        )  # [768, 384]: [Q01 | K01 | Q2;K2]
        bqk = np.zeros((128, 3), np.float32)
        bqk[:, 0] = bq_s[0:128]
        bqk[:, 1] = bk_s[0:128]
        bqk[0:64, 2] = bq_s[128:192]
        bqk[64:128, 2] = bk_s[128:192]
        wv = np.ascontiguousarray(Wv[hs].T.astype(np.float32))
        Wo_s = Wo[:, hs]                       # [768, 192]
        wo01 = np.ascontiguousarray(Wo_s[:, 0:128].T)   # [128, 768]
        wo2 = np.ascontiguousarray(Wo_s[:, 128:192].T)  # [64, 768]
        in_maps.append({
            "xT": np.ascontiguousarray(x[b].T).astype(bf),
            "wqk": np.ascontiguousarray(wqk).astype(bf),
            "bqk": np.ascontiguousarray(bqk),
            "wv": wv.astype(bf),
            "wo01": wo01.astype(bf),
            "wo2": wo2.astype(bf),
            "msk": mask.astype(bf),
        })
    return in_maps


_NC_CACHE = {}


def get_nc(t=T):
    if t not in _NC_CACHE:
        _NC_CACHE[t] = build_nc(t)
    return _NC_CACHE[t]


def run_cores(in_maps, t=T, trace=False, tmpdir=None):
    from concourse.bass_utils import run_bass_kernel_spmd

    nc = get_nc(t)
    return run_bass_kernel_spmd(
        nc, in_maps, list(range(N_CORES)), trace=trace, tmpdir=tmpdir
    )


def gather(results, x_shape, Wv_full, bv_full, Wo, bo):
    B, t, _ = x_shape
    out = np.zeros((B, t, C), np.float32)
    for c in range(N_CORES):
        b = c // (N_CORES // B)
        out[b] += results[c]["out"].T
    # bv folded out of the kernel: y@Wo.T picks up Wo @ bv per token
    out += (bo + Wo @ bv_full)[None, None, :]
    return out


def kernel(x, Wq, bq, Wk, bk, Wv, bv, Wo, bo, _trace=False, _tmpdir=None):
    x = np.asarray(x, dtype=np.float32)
    args = [np.asarray(a, dtype=np.float32) for a in (Wq, bq, Wk, bk, Wv, bv, Wo, bo)]
    Wq, bq, Wk, bk, Wv, bv, Wo, bo = args
    t = x.shape[1]
    in_maps = shard_inputs(x, Wq, bq, Wk, bk, Wv, bv, Wo, bo, t=t)
    res = run_cores(in_maps, t=t, trace=_trace, tmpdir=_tmpdir)
    out = gather(res.results, x.shape, Wv, bv, Wo, bo)
    kernel.last_result = res
    return out


# revision 36
# speedup vs baseline: 1.0205x; 1.0205x over previous
"""Multi-head causal attention (B=2, T=4096, C=768, H=12) on 8 TRN2 NeuronCores.

Sharding: 24 (batch, head) units -> 3 heads per core; cores 0-3 take batch 0,
cores 4-7 batch 1. Each core computes Q/K/V projections for its 3 heads, full-T
causal attention, and a partial output projection [C, T]. Host sums the 4
partials per batch and adds the output bias (+ Wo @ bv, folded out of the
device kernel).

Pipeline structure (per q-tile qi): the Q/K/V projections for tile qi+1 and
the out-projection for tile qi-1 are emitted as filler units spread through
qi's attention k-loop, so the PE never idles while the ACT engine (exp) is
the steady-state bottleneck, and the PE clock stays warm (HAM).

Device layouts (per core):
  xT    [768, T] bf16   x[b] transposed (c-major) - input
  Q, K  [d, t] bf16     head-pair tiles [128, 512] (partitions = 2x64 head dims)
  q2d, k2t [128, 512]   head-2 Q/K duplicated on both partition halves so
                        h2's QK matmuls for even/odd k-subtiles pair onto
                        disjoint PE row groups
  V     [t, d] bf16     per 128-row tile [128, 3*65] (65th col = ones -> denom)
  att^T [k, q]          QK^T computed transposed; heads 0/1 on PE row groups
                        0-1 / 2-3 run concurrently; h2 even/odd kt likewise
  exp   bf16            ACT Exp from PSUM; diagonal tiles column-trimmed;
                        causal mask applied as 0/1 multiply on the staircase
  y     [65, 512] psum  accum over k-tiles (row 64 = softmax denominator)
  yt01  [128, 512] bf16 normalized y for heads 0+1 stacked (K=128 out-proj)
  out   [768, T] f32    partial out-projection, c_out-major
"""

import ml_dtypes
import numpy as np

import concourse.bass as bass
import concourse.tile as tile
from concourse import bacc, mybir

F32 = mybir.dt.float32
BF16 = mybir.dt.bfloat16
AF = mybir.ActivationFunctionType

N_CORES = 8
T = 4096
C = 768
H = 12
D = 64
HPC = 3          # heads per core
QT = 512         # q-tile width (matmul N)
KT = 128         # k-tile width (partition dim)
NCH = C // 128   # 6 contraction chunks over C


def build_nc(t=T):
    nt = t // QT          # q/t tiles of 512
    nsub = t // KT        # t sub-tiles of 128

    nc = bacc.Bacc("TRN2", target_bir_lowering=False, debug=False)

    xT = nc.declare_dram_parameter("xT", [C, t], BF16, isOutput=False)
    wqk = nc.declare_dram_parameter("wqk", [C, 384], BF16, isOutput=False)
    bqk = nc.declare_dram_parameter("bqk", [128, 3], F32, isOutput=False)
    wv = nc.declare_dram_parameter("wv", [C, 192], BF16, isOutput=False)
    wo01 = nc.declare_dram_parameter("wo01", [128, C], BF16, isOutput=False)
    wo2 = nc.declare_dram_parameter("wo2", [64, C], BF16, isOutput=False)
    msk = nc.declare_dram_parameter("msk", [128, 4 * QT], BF16, isOutput=False)
    out = nc.declare_dram_parameter("out", [C, t], F32, isOutput=True)

    xT_r = xT.ap().rearrange("(a p) t -> p a t", p=128)
    wqk_r = wqk.ap().rearrange("(a p) m -> p a m", p=128)
    wv_r = wv.ap().rearrange("(a p) m -> p a m", p=128)

    with tile.TileContext(nc) as tc:
        with (
            tc.tile_pool(name="const", bufs=1) as const_pool,
            tc.tile_pool(name="xt", bufs=4 * NCH) as xp,
            tc.tile_pool(name="qp", bufs=nt) as qp,
            tc.tile_pool(name="kp", bufs=nt) as kp,
            tc.tile_pool(name="q2p", bufs=nt) as q2p,
            tc.tile_pool(name="k2p", bufs=nt) as k2p,
            tc.tile_pool(name="vp", bufs=nsub) as vp,
            tc.tile_pool(name="yp", bufs=nt) as yp,
            tc.tile_pool(name="ep", bufs=16) as ep,
            tc.tile_pool(name="op", bufs=8) as op,
            tc.tile_pool(name="sp", bufs=6) as sp,
            tc.tile_pool(name="ps_sc", bufs=2, space="PSUM") as ps_sc,
            tc.tile_pool(name="ps_b", bufs=4, space="PSUM") as ps_b,
        ):
            # ---- first x tile + hot constants first (head latency) ----
            xtiles = {}

            def emit_xt_dma(ti):
                xtc = []
                for ci in range(NCH):
                    xc = xp.tile([128, QT], BF16, tag="xt")
                    nc.sync.dma_start(
                        out=xc, in_=xT_r[:, ci, ti * QT:(ti + 1) * QT]
                    )
                    xtc.append(xc)
                xtiles[ti] = xtc

            wqk_sb = const_pool.tile([128, NCH, 384], BF16, tag="wqk")
            # m0 weights + x chunks first: first proj matmul gates the head
            nc.sync.dma_start(out=wqk_sb[:, :, 0:128], in_=wqk_r[:, :, 0:128])
            emit_xt_dma(0)
            for mm_ in (1, 2):
                nc.sync.dma_start(
                    out=wqk_sb[:, :, mm_ * 128:(mm_ + 1) * 128],
                    in_=wqk_r[:, :, mm_ * 128:(mm_ + 1) * 128],
                )
            bqk_sb = const_pool.tile([128, 3], F32, tag="bqk")
            nc.sync.dma_start(out=bqk_sb, in_=bqk.ap())
            wv_sb = const_pool.tile([128, NCH, 192], BF16, tag="wv")
            nc.sync.dma_start(out=wv_sb, in_=wv_r)
            mask_sb = const_pool.tile([128, 4, QT], BF16, tag="msk")
            msk_r = msk.ap().rearrange("p (o q) -> p o q", q=QT)
            nc.sync.dma_start(out=mask_sb, in_=msk_r)
            wo01_sb = const_pool.tile([128, C], BF16, tag="wo01")
            nc.sync.dma_start(out=wo01_sb, in_=wo01.ap())
            wo2_sb = const_pool.tile([64, C], BF16, tag="wo2")
            nc.sync.dma_start(out=wo2_sb, in_=wo2.ap())

            # HAM warm-up: ~5us of dependency-free dummy matmuls keep the
            # PE busy while the head DMAs stream, so real work starts at the
            # full 2.4 GHz clock instead of the cold 1.2 GHz half-rate.
            warm_in = const_pool.tile([128, QT], BF16, tag="warm")
            nc.vector.memset(warm_in, 0.0)
            warm_ps = ps_sc.tile([128, 2, QT], F32, tag="ps", name="warm_ps")
            for _ in range(24):
                nc.tensor.matmul(
                    warm_ps[:, 0, :], lhsT=warm_in[:, 0:128], rhs=warm_in,
                    start=True, stop=True,
                )

            q_t = [None] * nt
            k_t = [None] * nt
            q2_t = [None] * nt
            k2_t = [None] * nt
            v_t = [None] * (4 * nt)
            yt01_t = [None] * nt
            yt2_t = [None] * nt

            # ---------- emission units ----------

            def proj_m01(ti, m):
                # m = 0: Q heads 0/1; m = 1: K heads 0/1. M = 128.
                ps = ps_b.tile([128, QT], F32, tag="psb")
                for ci in range(NCH):
                    nc.tensor.matmul(
                        ps,
                        lhsT=wqk_sb[:, ci, m * 128:(m + 1) * 128],
                        rhs=xtiles[ti][ci],
                        start=(ci == 0),
                        stop=(ci == NCH - 1),
                    )
                pool = (qp, kp)[m]
                dst = pool.tile([128, QT], BF16, tag=("q", "k")[m])
                nc.vector.tensor_scalar_add(dst, ps, bqk_sb[:, m:m + 1])
                if m == 0:
                    q_t[ti] = dst
                else:
                    k_t[ti] = dst

            def proj_m2(ti):
                # [Q2; K2] stacked: q2 on partitions 0-63, k2 on 64-127.
                ps = ps_b.tile([128, QT], F32, tag="psb")
                for ci in range(NCH):
                    nc.tensor.matmul(
                        ps,
                        lhsT=wqk_sb[:, ci, 256:384],
                        rhs=xtiles[ti][ci],
                        start=(ci == 0),
                        stop=(ci == NCH - 1),
                    )
                q2d = q2p.tile([128, QT], BF16, tag="q2")
                k2t = k2p.tile([128, QT], BF16, tag="k2")
                nc.vector.tensor_scalar_add(
                    q2d[0:64, :], ps[0:64, :], bqk_sb[0:64, 2:3]
                )
                nc.vector.tensor_scalar_add(
                    k2t[64:128, :], ps[64:128, :], bqk_sb[64:128, 2:3]
                )
                # Duplicate to the other partition half (DMA partition move,
                # off the PE critical path) so h2's QK matmuls pair onto
                # disjoint PE row groups.
                nc.sync.dma_start(out=q2d[64:128, :], in_=q2d[0:64, :])
                nc.sync.dma_start(out=k2t[0:64, :], in_=k2t[64:128, :])
                q2_t[ti] = q2d
                k2_t[ti] = k2t

            def proj_v(ti, si):
                psv = ps_b.tile([128, 192], F32, tag="psb")
                for ci in range(NCH):
                    nc.tensor.matmul(
                        psv,
                        lhsT=xtiles[ti][ci][:, si * 128:(si + 1) * 128],
                        rhs=wv_sb[:, ci, :],
                        start=(ci == 0),
                        stop=(ci == NCH - 1),
                    )
                vt = vp.tile([128, HPC * 65], BF16, tag="v")
                vt_r = vt.rearrange("p (h e) -> p h e", e=65)
                nc.vector.memset(vt_r[:, :, 64:65], 1.0)
                nc.vector.tensor_copy(
                    vt_r[:, :, 0:64],
                    psv[:, 0:HPC * 64].rearrange("p (h e) -> p h e", e=64),
                )
                v_t[4 * ti + si] = vt

            def proj_units(ti):
                yield lambda: proj_m01(ti, 0)
                yield lambda: proj_m01(ti, 1)
                yield lambda: proj_m2(ti)
                for si in range(4):
                    yield lambda si=si: proj_v(ti, si)

            def outproj_mo(qi, mo, wo2_first=False):
                # For the last q-tile the wo2 matmul goes first: yt2 is
                # normalized first there, so the out-projection overlaps the
                # rest of the normalize chain instead of waiting for yt01.
                ps = ps_b.tile([128, QT], F32, tag="psb")
                pair = [
                    (wo01_sb, yt01_t[qi]),
                    (wo2_sb, yt2_t[qi]),
                ]
                if wo2_first:
                    pair.reverse()
                for i, (w, y) in enumerate(pair):
                    nc.tensor.matmul(
                        ps,
                        lhsT=w[:, mo * 128:(mo + 1) * 128],
                        rhs=y,
                        start=(i == 0),
                        stop=(i == 1),
                    )
                ot = op.tile([128, QT], F32, tag="o")
                nc.vector.tensor_copy(ot, ps)
                nc.sync.dma_start(
                    out=out.ap()[mo * 128:(mo + 1) * 128,
                                 qi * QT:(qi + 1) * QT],
                    in_=ot,
                )

            def outproj_units(qi):
                last = qi == nt - 1
                for mo in range(NCH):
                    yield lambda mo=mo: outproj_mo(qi, mo, wo2_first=last)

            def normalize(qi, h, y_ps, dst_ap):
                # y_ps row 64 = denominator. DVE-copy it straight to SBUF
                # partition 0 (64->0 is a quadrant-aligned cross-quadrant
                # move), reciprocal on 1 partition, broadcast on GpSimd,
                # multiply on DVE.
                den = sp.tile([1, QT], F32, tag="den")
                nc.vector.tensor_copy(den, y_ps[64:65, :])
                rec = sp.tile([1, QT], F32, tag="rec")
                nc.vector.reciprocal_approx_fast(rec, den)
                bc = sp.tile([64, QT], F32, tag="bcs")
                nc.gpsimd.partition_broadcast(bc, rec[0:1, :])
                nc.vector.tensor_mul(dst_ap, y_ps[0:64, :], bc)

            # ---------- main pipeline ----------
            filler = []          # queue of pending PE filler units

            def emit_fillers(k):
                for _ in range(k):
                    if filler:
                        filler.pop(0)()

            # prologue: only Q01/K01 of tile 0 up front -- att(0)'s QK/exp
            # can then start immediately; m2 + V projections arrive as the
            # first fillers, just in time for h2's QK and the deferred attV.
            proj_m01(0, 0)
            proj_m01(0, 1)
            filler.append(lambda: proj_m2(0))
            for si in range(4):
                filler.append(lambda si=si: proj_v(0, si))
            if nt > 1:
                emit_xt_dma(1)
                filler.extend(proj_units(1))

            for qi in range(nt):
                n_k = 4 * qi + 4
                q0_ap = q_t[qi][0:64, :]
                q1_ap = q_t[qi][64:128, :]
                y0 = ps_b.tile([128, QT], F32, tag="psb")
                y1 = ps_b.tile([128, QT], F32, tag="psb")
                y2 = ps_b.tile([128, QT], F32, tag="psb")
                # filler spacing: spread pending units across this kt loop;
                # front-load a chunk so the PE has work queued ahead of the
                # first attV flushes while normalize(qi-1) releases y slots
                n_fill = len(filler)
                done_fill = 0
                emit_fillers(max(0, n_fill // 4))
                done_fill = min(n_fill, max(0, n_fill // 4))

                # attV is deferred by one k-tile (h01) / one pair (h2) so the
                # exp it consumes finished during the previous iteration's PE
                # work -- avoids a PE pipeline restart per k-tile.
                pq01 = []   # FIFO of (kt, et, lo), flushed at depth 2
                pq2 = []    # FIFO of (kt0, et2), flushed at depth 2

                def flush01_one():
                    pkt, pet, plo = pq01.pop(0)
                    nc.tensor.matmul(
                        y0[0:65, plo:QT],
                        lhsT=v_t[pkt][:, 0:65],
                        rhs=pet[:, 0, plo:QT],
                        start=(pkt == 0),
                        stop=(pkt == n_k - 1),
                    )
                    nc.tensor.matmul(
                        y1[0:65, plo:QT],
                        lhsT=v_t[pkt][:, 65:130],
                        rhs=pet[:, 1, plo:QT],
                        start=(pkt == 0),
                        stop=(pkt == n_k - 1),
                    )

                def flush2_one():
                    pkt0, pet2 = pq2.pop(0)
                    for u in (0, 1):
                        ku = pkt0 + u
                        lou = max(0, ku - 4 * qi) * 128
                        nc.tensor.matmul(
                            y2[0:65, lou:QT],
                            lhsT=v_t[ku][:, 130:195],
                            rhs=pet2[:, u, lou:QT],
                            start=(ku == 0),
                            stop=(ku == n_k - 1),
                        )

                for kt in range(n_k):
                    # fillers first: they absorb the wait for the score-slot
                    # release (ACT-bound cadence) instead of a PE restart
                    want = ((kt + 1) * n_fill) // n_k
                    emit_fillers(want - done_fill)
                    done_fill = max(done_fill, want)

                    tj, tcol = kt // 4, (kt % 4) * 128
                    o = kt - 4 * qi
                    lo = max(0, o) * 128   # first unmasked column
                    aps = ps_sc.tile([128, 2, QT], F32, tag="ps")
                    nc.tensor.matmul(
                        aps[:, 0, lo:QT], lhsT=k_t[tj][0:64, tcol:tcol + 128],
                        rhs=q0_ap[:, lo:QT], start=True, stop=True,
                    )
                    nc.tensor.matmul(
                        aps[:, 1, lo:QT], lhsT=k_t[tj][64:128, tcol:tcol + 128],
                        rhs=q1_ap[:, lo:QT], start=True, stop=True,
                    )
                    et = ep.tile([128, 2, QT], BF16, tag="e")
                    nc.scalar.activation(et[:, :, lo:QT], aps[:, :, lo:QT], AF.Exp)
                    if o >= 0:
                        nc.vector.tensor_mul(
                            et[:, :, lo:QT], et[:, :, lo:QT],
                            mask_sb[:, o:o + 1, lo:QT].to_broadcast(
                                [128, 2, QT - lo]
                            ),
                        )
                    pq01.append((kt, et, lo))
                    while len(pq01) > 2:
                        flush01_one()

                    # ---- head 2: one pair of k-tiles per exp ----
                    if kt % 2 == 1:
                        kt0 = kt - 1
                        o0 = max(0, kt0 - 4 * qi)
                        lo0 = o0 * 128
                        aps2 = ps_sc.tile([128, 2, QT], F32, tag="ps")
                        for u in (0, 1):
                            ku = kt0 + u
                            tju, tcu = ku // 4, (ku % 4) * 128
                            base = 64 * (ku % 2)
                            # both slots cover [lo0:QT] so exp never reads
                            # never-written PSUM; mask zeroes the staircase
                            nc.tensor.matmul(
                                aps2[:, u, lo0:QT],
                                lhsT=k2_t[tju][base:base + 64, tcu:tcu + 128],
                                rhs=q2_t[qi][base:base + 64, lo0:QT],
                                start=True, stop=True,
                            )
                        et2 = ep.tile([128, 2, QT], BF16, tag="e")
                        nc.scalar.activation(
                            et2[:, :, lo0:QT], aps2[:, :, lo0:QT], AF.Exp
                        )
                        for u in (0, 1):
                            ou = kt0 + u - 4 * qi
                            if ou >= 0:
                                nc.vector.tensor_mul(
                                    et2[:, u, lo0:QT], et2[:, u, lo0:QT],
                                    mask_sb[:, ou, lo0:QT],
                                )
                        pq2.append((kt0, et2))
                        while len(pq2) > 2:
                            flush2_one()

                while pq01:
                    flush01_one()
                while pq2:
                    flush2_one()

                # ---- normalize ----
                yt01 = yp.tile([128, QT], BF16, tag="y01")
                yt2 = yp.tile([64, QT], BF16, tag="y2")
                if qi == nt - 1:
                    normalize(qi, 2, y2, yt2)
                normalize(qi, 0, y0, yt01[0:64, :])
                # h1 writes cross-quadrant (banks 0,1 -> quadrants 2,3)
                normalize(qi, 1, y1, yt01[64:128, :])
                if qi != nt - 1:
                    normalize(qi, 2, y2, yt2)
                yt01_t[qi] = yt01
                yt2_t[qi] = yt2

                emit_fillers(len(filler))

                # refill the filler queue for the next iteration
                if qi + 2 < nt:
                    emit_xt_dma(qi + 2)
                    filler.extend(proj_units(qi + 2))
                filler.extend(outproj_units(qi))

            # drain remaining fillers (last outproj)
            emit_fillers(len(filler))

    nc.compile()
    return nc


def make_mask():
    i = np.arange(128)[:, None]
    j = np.arange(QT)[None, :]
    m = np.zeros((128, 4 * QT), np.float32)
    for o in range(4):
        m[:, o * QT:(o + 1) * QT] = (j >= o * 128 + i)
    return m


def shard_inputs(x, Wq, bq, Wk, bk, Wv, bv, Wo, bo, t=T):
    """Build per-core in_maps."""
    s = 1.0 / np.sqrt(D)
    mask = make_mask()
    bf = ml_dtypes.bfloat16
    in_maps = []
    for c in range(N_CORES):
        b = c // (N_CORES // x.shape[0])
        h0 = HPC * (c % 4)
        hs = slice(h0 * D, (h0 + HPC) * D)
        Wq_s = (Wq[hs] * s).astype(np.float32)
        bq_s = (bq[hs] * s).astype(np.float32)
        Wk_s, bk_s = Wk[hs], bk[hs]
        wqk = np.concatenate(
            [Wq_s[0:128].T, Wk_s[0:128].T, Wq_s[128:192].T, Wk_s[128:192].T],
            axis=1,
        )  # [768, 384]: [Q01 | K01 | Q2;K2]
        bqk = np.zeros((128, 3), np.float32)
        bqk[:, 0] = bq_s[0:128]
        bqk[:, 1] = bk_s[0:128]
        bqk[0:64, 2] = bq_s[128:192]
        bqk[64:128, 2] = bk_s[128:192]
        wv = np.ascontiguousarray(Wv[hs].T.astype(np.float32))
        Wo_s = Wo[:, hs]                       # [768, 192]
        wo01 = np.ascontiguousarray(Wo_s[:, 0:128].T)   # [128, 768]
        wo2 = np.ascontiguousarray(Wo_s[:, 128:192].T)  # [64, 768]
        in_maps.append({
            "xT": np.ascontiguousarray(x[b].T).astype(bf),
            "wqk": np.ascontiguousarray(wqk).astype(bf),
            "bqk": np.ascontiguousarray(bqk),
            "wv": wv.astype(bf),
            "wo01": wo01.astype(bf),
            "wo2": wo2.astype(bf),
            "msk": mask.astype(bf),
        })
    return in_maps


_NC_CACHE = {}


def get_nc(t=T):
    if t not in _NC_CACHE:
        _NC_CACHE[t] = build_nc(t)
    return _NC_CACHE[t]


def run_cores(in_maps, t=T, trace=False, tmpdir=None):
    from concourse.bass_utils import run_bass_kernel_spmd

    nc = get_nc(t)
    return run_bass_kernel_spmd(
        nc, in_maps, list(range(N_CORES)), trace=trace, tmpdir=tmpdir
    )


def gather(results, x_shape, Wv_full, bv_full, Wo, bo):
    B, t, _ = x_shape
    out = np.zeros((B, t, C), np.float32)
    for c in range(N_CORES):
        b = c // (N_CORES // B)
        out[b] += results[c]["out"].T
    # bv folded out of the kernel: y@Wo.T picks up Wo @ bv per token
    out += (bo + Wo @ bv_full)[None, None, :]
    return out


def kernel(x, Wq, bq, Wk, bk, Wv, bv, Wo, bo, _trace=False, _tmpdir=None):
    x = np.asarray(x, dtype=np.float32)
    args = [np.asarray(a, dtype=np.float32) for a in (Wq, bq, Wk, bk, Wv, bv, Wo, bo)]
    Wq, bq, Wk, bk, Wv, bv, Wo, bo = args
    t = x.shape[1]
    in_maps = shard_inputs(x, Wq, bq, Wk, bk, Wv, bv, Wo, bo, t=t)
    res = run_cores(in_maps, t=t, trace=_trace, tmpdir=_tmpdir)
    out = gather(res.results, x.shape, Wv, bv, Wo, bo)
    kernel.last_result = res
    return out


# revision 37
# speedup vs baseline: 1.0301x; 1.0094x over previous
"""Multi-head causal attention (B=2, T=4096, C=768, H=12) on 8 TRN2 NeuronCores.

Sharding: 24 (batch, head) units -> 3 heads per core; cores 0-3 take batch 0,
cores 4-7 batch 1. Each core computes Q/K/V projections for its 3 heads, full-T
causal attention, and a partial output projection [C, T]. Host sums the 4
partials per batch and adds the output bias (+ Wo @ bv, folded out of the
device kernel).

Pipeline structure (per q-tile qi): the Q/K/V projections for tile qi+1 and
the out-projection for tile qi-1 are emitted as filler units spread through
qi's attention k-loop, so the PE never idles while the ACT engine (exp) is
the steady-state bottleneck, and the PE clock stays warm (HAM).

Device layouts (per core):
  xT    [768, T] bf16   x[b] transposed (c-major) - input
  Q, K  [d, t] bf16     head-pair tiles [128, 512] (partitions = 2x64 head dims)
  q2d, k2t [128, 512]   head-2 Q/K duplicated on both partition halves so
                        h2's QK matmuls for even/odd k-subtiles pair onto
                        disjoint PE row groups
  V     [t, d] bf16     per 128-row tile [128, 3*65] (65th col = ones -> denom)
  att^T [k, q]          QK^T computed transposed; heads 0/1 on PE row groups
                        0-1 / 2-3 run concurrently; h2 even/odd kt likewise
  exp   bf16            ACT Exp from PSUM; diagonal tiles column-trimmed;
                        causal mask applied as 0/1 multiply on the staircase
  y     [65, 512] psum  accum over k-tiles (row 64 = softmax denominator)
  yt01  [128, 512] bf16 normalized y for heads 0+1 stacked (K=128 out-proj)
  out   [768, T] f32    partial out-projection, c_out-major
"""

import ml_dtypes
import numpy as np

import concourse.bass as bass
import concourse.tile as tile
from concourse import bacc, mybir

F32 = mybir.dt.float32
BF16 = mybir.dt.bfloat16
AF = mybir.ActivationFunctionType

N_CORES = 8
T = 4096
C = 768
H = 12
D = 64
HPC = 3          # heads per core
QT = 512         # q-tile width (matmul N)
KT = 128         # k-tile width (partition dim)
NCH = C // 128   # 6 contraction chunks over C


def build_nc(t=T):
    nt = t // QT          # q/t tiles of 512
    nsub = t // KT        # t sub-tiles of 128

    nc = bacc.Bacc("TRN2", target_bir_lowering=False, debug=False)

    xT = nc.declare_dram_parameter("xT", [C, t], BF16, isOutput=False)
    wqk = nc.declare_dram_parameter("wqk", [C, 384], BF16, isOutput=False)
    bqk = nc.declare_dram_parameter("bqk", [128, 3], F32, isOutput=False)
    wv = nc.declare_dram_parameter("wv", [C, 192], BF16, isOutput=False)
    wo01 = nc.declare_dram_parameter("wo01", [128, C], BF16, isOutput=False)
    wo2 = nc.declare_dram_parameter("wo2", [64, C], BF16, isOutput=False)
    msk = nc.declare_dram_parameter("msk", [128, 4 * QT], BF16, isOutput=False)
    out = nc.declare_dram_parameter("out", [C, t], F32, isOutput=True)

    xT_r = xT.ap().rearrange("(a p) t -> p a t", p=128)
    wqk_r = wqk.ap().rearrange("(a p) m -> p a m", p=128)
    wv_r = wv.ap().rearrange("(a p) m -> p a m", p=128)

    with tile.TileContext(nc) as tc:
        with (
            tc.tile_pool(name="const", bufs=1) as const_pool,
            tc.tile_pool(name="xt", bufs=4 * NCH) as xp,
            tc.tile_pool(name="qp", bufs=nt) as qp,
            tc.tile_pool(name="kp", bufs=nt) as kp,
            tc.tile_pool(name="q2p", bufs=nt) as q2p,
            tc.tile_pool(name="k2p", bufs=nt) as k2p,
            tc.tile_pool(name="vp", bufs=nsub) as vp,
            tc.tile_pool(name="yp", bufs=nt) as yp,
            tc.tile_pool(name="ep", bufs=20) as ep,
            tc.tile_pool(name="op", bufs=8) as op,
            tc.tile_pool(name="sp", bufs=6) as sp,
            tc.tile_pool(name="ps_sc", bufs=2, space="PSUM") as ps_sc,
            tc.tile_pool(name="ps_b", bufs=4, space="PSUM") as ps_b,
        ):
            # ---- first x tile + hot constants first (head latency) ----
            xtiles = {}

            def emit_xt_dma(ti):
                xtc = []
                for ci in range(NCH):
                    xc = xp.tile([128, QT], BF16, tag="xt")
                    nc.sync.dma_start(
                        out=xc, in_=xT_r[:, ci, ti * QT:(ti + 1) * QT]
                    )
                    xtc.append(xc)
                xtiles[ti] = xtc

            wqk_sb = const_pool.tile([128, NCH, 384], BF16, tag="wqk")
            # m0 weights + x chunks first: first proj matmul gates the head
            nc.sync.dma_start(out=wqk_sb[:, :, 0:128], in_=wqk_r[:, :, 0:128])
            emit_xt_dma(0)
            for mm_ in (1, 2):
                nc.sync.dma_start(
                    out=wqk_sb[:, :, mm_ * 128:(mm_ + 1) * 128],
                    in_=wqk_r[:, :, mm_ * 128:(mm_ + 1) * 128],
                )
            bqk_sb = const_pool.tile([128, 3], F32, tag="bqk")
            nc.sync.dma_start(out=bqk_sb, in_=bqk.ap())
            wv_sb = const_pool.tile([128, NCH, 192], BF16, tag="wv")
            nc.sync.dma_start(out=wv_sb, in_=wv_r)
            mask_sb = const_pool.tile([128, 4, QT], BF16, tag="msk")
            msk_r = msk.ap().rearrange("p (o q) -> p o q", q=QT)
            nc.sync.dma_start(out=mask_sb, in_=msk_r)
            wo01_sb = const_pool.tile([128, C], BF16, tag="wo01")
            nc.sync.dma_start(out=wo01_sb, in_=wo01.ap())
            wo2_sb = const_pool.tile([64, C], BF16, tag="wo2")
            nc.sync.dma_start(out=wo2_sb, in_=wo2.ap())

            # HAM warm-up: ~5us of dependency-free dummy matmuls keep the
            # PE busy while the head DMAs stream, so real work starts at the
            # full 2.4 GHz clock instead of the cold 1.2 GHz half-rate.
            warm_in = const_pool.tile([128, QT], BF16, tag="warm")
            nc.vector.memset(warm_in, 0.0)
            warm_ps = ps_sc.tile([128, 2, QT], F32, tag="ps", name="warm_ps")
            for _ in range(24):
                nc.tensor.matmul(
                    warm_ps[:, 0, :], lhsT=warm_in[:, 0:128], rhs=warm_in,
                    start=True, stop=True,
                )

            q_t = [None] * nt
            k_t = [None] * nt
            q2_t = [None] * nt
            k2_t = [None] * nt
            v_t = [None] * (4 * nt)
            yt01_t = [None] * nt
            yt2_t = [None] * nt

            # ---------- emission units ----------

            def proj_m01(ti, m):
                # m = 0: Q heads 0/1; m = 1: K heads 0/1. M = 128.
                ps = ps_b.tile([128, QT], F32, tag="psb")
                for ci in range(NCH):
                    nc.tensor.matmul(
                        ps,
                        lhsT=wqk_sb[:, ci, m * 128:(m + 1) * 128],
                        rhs=xtiles[ti][ci],
                        start=(ci == 0),
                        stop=(ci == NCH - 1),
                    )
                pool = (qp, kp)[m]
                dst = pool.tile([128, QT], BF16, tag=("q", "k")[m])
                nc.vector.tensor_scalar_add(dst, ps, bqk_sb[:, m:m + 1])
                if m == 0:
                    q_t[ti] = dst
                else:
                    k_t[ti] = dst

            def proj_m2(ti):
                # [Q2; K2] stacked: q2 on partitions 0-63, k2 on 64-127.
                ps = ps_b.tile([128, QT], F32, tag="psb")
                for ci in range(NCH):
                    nc.tensor.matmul(
                        ps,
                        lhsT=wqk_sb[:, ci, 256:384],
                        rhs=xtiles[ti][ci],
                        start=(ci == 0),
                        stop=(ci == NCH - 1),
                    )
                q2d = q2p.tile([128, QT], BF16, tag="q2")
                k2t = k2p.tile([128, QT], BF16, tag="k2")
                nc.vector.tensor_scalar_add(
                    q2d[0:64, :], ps[0:64, :], bqk_sb[0:64, 2:3]
                )
                nc.vector.tensor_scalar_add(
                    k2t[64:128, :], ps[64:128, :], bqk_sb[64:128, 2:3]
                )
                # Duplicate to the other partition half (DMA partition move,
                # off the PE critical path) so h2's QK matmuls pair onto
                # disjoint PE row groups.
                nc.sync.dma_start(out=q2d[64:128, :], in_=q2d[0:64, :])
                nc.sync.dma_start(out=k2t[0:64, :], in_=k2t[64:128, :])
                q2_t[ti] = q2d
                k2_t[ti] = k2t

            def proj_v(ti, si):
                psv = ps_b.tile([128, 192], F32, tag="psb")
                for ci in range(NCH):
                    nc.tensor.matmul(
                        psv,
                        lhsT=xtiles[ti][ci][:, si * 128:(si + 1) * 128],
                        rhs=wv_sb[:, ci, :],
                        start=(ci == 0),
                        stop=(ci == NCH - 1),
                    )
                vt = vp.tile([128, HPC * 65], BF16, tag="v")
                vt_r = vt.rearrange("p (h e) -> p h e", e=65)
                nc.vector.memset(vt_r[:, :, 64:65], 1.0)
                nc.vector.tensor_copy(
                    vt_r[:, :, 0:64],
                    psv[:, 0:HPC * 64].rearrange("p (h e) -> p h e", e=64),
                )
                v_t[4 * ti + si] = vt

            def proj_units(ti):
                yield lambda: proj_m01(ti, 0)
                yield lambda: proj_m01(ti, 1)
                yield lambda: proj_m2(ti)
                for si in range(4):
                    yield lambda si=si: proj_v(ti, si)

            def outproj_mo(qi, mo, wo2_first=False):
                # For the last q-tile the wo2 matmul goes first: yt2 is
                # normalized first there, so the out-projection overlaps the
                # rest of the normalize chain instead of waiting for yt01.
                ps = ps_b.tile([128, QT], F32, tag="psb")
                pair = [
                    (wo01_sb, yt01_t[qi]),
                    (wo2_sb, yt2_t[qi]),
                ]
                if wo2_first:
                    pair.reverse()
                for i, (w, y) in enumerate(pair):
                    nc.tensor.matmul(
                        ps,
                        lhsT=w[:, mo * 128:(mo + 1) * 128],
                        rhs=y,
                        start=(i == 0),
                        stop=(i == 1),
                    )
                ot = op.tile([128, QT], F32, tag="o")
                nc.vector.tensor_copy(ot, ps)
                nc.sync.dma_start(
                    out=out.ap()[mo * 128:(mo + 1) * 128,
                                 qi * QT:(qi + 1) * QT],
                    in_=ot,
                )

            def outproj_units(qi):
                last = qi == nt - 1
                for mo in range(NCH):
                    yield lambda mo=mo: outproj_mo(qi, mo, wo2_first=last)

            def normalize(qi, h, y_ps, dst_ap):
                # y_ps row 64 = denominator. DVE-copy it straight to SBUF
                # partition 0 (64->0 is a quadrant-aligned cross-quadrant
                # move), reciprocal on 1 partition, broadcast on GpSimd,
                # multiply on DVE.
                den = sp.tile([1, QT], F32, tag="den")
                nc.vector.tensor_copy(den, y_ps[64:65, :])
                rec = sp.tile([1, QT], F32, tag="rec")
                nc.vector.reciprocal_approx_fast(rec, den)
                bc = sp.tile([64, QT], F32, tag="bcs")
                nc.gpsimd.partition_broadcast(bc, rec[0:1, :])
                nc.vector.tensor_mul(dst_ap, y_ps[0:64, :], bc)

            # ---------- main pipeline ----------
            filler = []          # queue of pending PE filler units

            def emit_fillers(k):
                for _ in range(k):
                    if filler:
                        filler.pop(0)()

            # prologue: only Q01/K01 of tile 0 up front -- att(0)'s QK/exp
            # can then start immediately; m2 + V projections arrive as the
            # first fillers, just in time for h2's QK and the deferred attV.
            proj_m01(0, 0)
            proj_m01(0, 1)
            filler.append(lambda: proj_m2(0))
            for si in range(4):
                filler.append(lambda si=si: proj_v(0, si))
            if nt > 1:
                emit_xt_dma(1)
                filler.extend(proj_units(1))

            for qi in range(nt):
                n_k = 4 * qi + 4
                q0_ap = q_t[qi][0:64, :]
                q1_ap = q_t[qi][64:128, :]
                y0 = ps_b.tile([128, QT], F32, tag="psb")
                y1 = ps_b.tile([128, QT], F32, tag="psb")
                y2 = ps_b.tile([128, QT], F32, tag="psb")
                # filler spacing: spread pending units across this kt loop;
                # front-load a chunk so the PE has work queued ahead of the
                # first attV flushes while normalize(qi-1) releases y slots
                n_fill = len(filler)
                done_fill = 0
                emit_fillers(max(0, n_fill // 4))
                done_fill = min(n_fill, max(0, n_fill // 4))

                # attV is deferred by one k-tile (h01) / one pair (h2) so the
                # exp it consumes finished during the previous iteration's PE
                # work -- avoids a PE pipeline restart per k-tile.
                pq01 = []   # FIFO of (kt, et, lo), flushed at depth 3
                pq2 = []    # FIFO of (kt0, et2), flushed at depth 3

                def flush01_one():
                    pkt, pet, plo = pq01.pop(0)
                    nc.tensor.matmul(
                        y0[0:65, plo:QT],
                        lhsT=v_t[pkt][:, 0:65],
                        rhs=pet[:, 0, plo:QT],
                        start=(pkt == 0),
                        stop=(pkt == n_k - 1),
                    )
                    nc.tensor.matmul(
                        y1[0:65, plo:QT],
                        lhsT=v_t[pkt][:, 65:130],
                        rhs=pet[:, 1, plo:QT],
                        start=(pkt == 0),
                        stop=(pkt == n_k - 1),
                    )

                def flush2_one():
                    pkt0, pet2 = pq2.pop(0)
                    for u in (0, 1):
                        ku = pkt0 + u
                        lou = max(0, ku - 4 * qi) * 128
                        nc.tensor.matmul(
                            y2[0:65, lou:QT],
                            lhsT=v_t[ku][:, 130:195],
                            rhs=pet2[:, u, lou:QT],
                            start=(ku == 0),
                            stop=(ku == n_k - 1),
                        )

                for kt in range(n_k):
                    # fillers first: they absorb the wait for the score-slot
                    # release (ACT-bound cadence) instead of a PE restart
                    want = ((kt + 1) * n_fill) // n_k
                    emit_fillers(want - done_fill)
                    done_fill = max(done_fill, want)

                    tj, tcol = kt // 4, (kt % 4) * 128
                    o = kt - 4 * qi
                    lo = max(0, o) * 128   # first unmasked column
                    aps = ps_sc.tile([128, 2, QT], F32, tag="ps")
                    nc.tensor.matmul(
                        aps[:, 0, lo:QT], lhsT=k_t[tj][0:64, tcol:tcol + 128],
                        rhs=q0_ap[:, lo:QT], start=True, stop=True,
                    )
                    nc.tensor.matmul(
                        aps[:, 1, lo:QT], lhsT=k_t[tj][64:128, tcol:tcol + 128],
                        rhs=q1_ap[:, lo:QT], start=True, stop=True,
                    )
                    et = ep.tile([128, 2, QT], BF16, tag="e")
                    nc.scalar.activation(et[:, :, lo:QT], aps[:, :, lo:QT], AF.Exp)
                    if o >= 0:
                        nc.vector.tensor_mul(
                            et[:, :, lo:QT], et[:, :, lo:QT],
                            mask_sb[:, o:o + 1, lo:QT].to_broadcast(
                                [128, 2, QT - lo]
                            ),
                        )
                    pq01.append((kt, et, lo))
                    while len(pq01) > 3:
                        flush01_one()

                    # ---- head 2: one pair of k-tiles per exp ----
                    if kt % 2 == 1:
                        kt0 = kt - 1
                        o0 = max(0, kt0 - 4 * qi)
                        lo0 = o0 * 128
                        aps2 = ps_sc.tile([128, 2, QT], F32, tag="ps")
                        for u in (0, 1):
                            ku = kt0 + u
                            tju, tcu = ku // 4, (ku % 4) * 128
                            base = 64 * (ku % 2)
                            # both slots cover [lo0:QT] so exp never reads
                            # never-written PSUM; mask zeroes the staircase
                            nc.tensor.matmul(
                                aps2[:, u, lo0:QT],
                                lhsT=k2_t[tju][base:base + 64, tcu:tcu + 128],
                                rhs=q2_t[qi][base:base + 64, lo0:QT],
                                start=True, stop=True,
                            )
                        et2 = ep.tile([128, 2, QT], BF16, tag="e")
                        nc.scalar.activation(
                            et2[:, :, lo0:QT], aps2[:, :, lo0:QT], AF.Exp
                        )
                        for u in (0, 1):
                            ou = kt0 + u - 4 * qi
                            if ou >= 0:
                                nc.vector.tensor_mul(
                                    et2[:, u, lo0:QT], et2[:, u, lo0:QT],
                                    mask_sb[:, ou, lo0:QT],
                                )
                        pq2.append((kt0, et2))
                        while len(pq2) > 3:
                            flush2_one()

                while pq01:
                    flush01_one()
                while pq2:
                    flush2_one()

                # ---- normalize ----
                yt01 = yp.tile([128, QT], BF16, tag="y01")
                yt2 = yp.tile([64, QT], BF16, tag="y2")
                if qi == nt - 1:
                    normalize(qi, 2, y2, yt2)
                normalize(qi, 0, y0, yt01[0:64, :])
                # h1 writes cross-quadrant (banks 0,1 -> quadrants 2,3)
                normalize(qi, 1, y1, yt01[64:128, :])
                if qi != nt - 1:
                    normalize(qi, 2, y2, yt2)
                yt01_t[qi] = yt01
                yt2_t[qi] = yt2

                emit_fillers(len(filler))

                # refill the filler queue for the next iteration
                if qi + 2 < nt:
                    emit_xt_dma(qi + 2)
                    filler.extend(proj_units(qi + 2))
                filler.extend(outproj_units(qi))

            # drain remaining fillers (last outproj)
            emit_fillers(len(filler))

    nc.compile()
    return nc


def make_mask():
    i = np.arange(128)[:, None]
    j = np.arange(QT)[None, :]
    m = np.zeros((128, 4 * QT), np.float32)
    for o in range(4):
        m[:, o * QT:(o + 1) * QT] = (j >= o * 128 + i)
    return m


def shard_inputs(x, Wq, bq, Wk, bk, Wv, bv, Wo, bo, t=T):
    """Build per-core in_maps."""
    s = 1.0 / np.sqrt(D)
    mask = make_mask()
    bf = ml_dtypes.bfloat16
    in_maps = []
    for c in range(N_CORES):
        b = c // (N_CORES // x.shape[0])
        h0 = HPC * (c % 4)
        hs = slice(h0 * D, (h0 + HPC) * D)
        Wq_s = (Wq[hs] * s).astype(np.float32)
        bq_s = (bq[hs] * s).astype(np.float32)
        Wk_s, bk_s = Wk[hs], bk[hs]
        wqk = np.concatenate(
            [Wq_s[0:128].T, Wk_s[0:128].T, Wq_s[128:192].T, Wk_s[128:192].T],
            axis=1,
        )  # [768, 384]: [Q01 | K01 | Q2;K2]
        bqk = np.zeros((128, 3), np.float32)
        bqk[:, 0] = bq_s[0:128]
        bqk[:, 1] = bk_s[0:128]
        bqk[0:64, 2] = bq_s[128:192]
        bqk[64:128, 2] = bk_s[128:192]
        wv = np.ascontiguousarray(Wv[hs].T.astype(np.float32))
        Wo_s = Wo[:, hs]                       # [768, 192]
        wo01 = np.ascontiguousarray(Wo_s[:, 0:128].T)   # [128, 768]
        wo2 = np.ascontiguousarray(Wo_s[:, 128:192].T)  # [64, 768]
        in_maps.append({
            "xT": np.ascontiguousarray(x[b].T).astype(bf),
            "wqk": np.ascontiguousarray(wqk).astype(bf),
            "bqk": np.ascontiguousarray(bqk),
            "wv": wv.astype(bf),
            "wo01": wo01.astype(bf),
            "wo2": wo2.astype(bf),
            "msk": mask.astype(bf),
        })
    return in_maps


_NC_CACHE = {}


def get_nc(t=T):
    if t not in _NC_CACHE:
        _NC_CACHE[t] = build_nc(t)
    return _NC_CACHE[t]


def run_cores(in_maps, t=T, trace=False, tmpdir=None):
    from concourse.bass_utils import run_bass_kernel_spmd

    nc = get_nc(t)
    return run_bass_kernel_spmd(
        nc, in_maps, list(range(N_CORES)), trace=trace, tmpdir=tmpdir
    )


def gather(results, x_shape, Wv_full, bv_full, Wo, bo):
    B, t, _ = x_shape
    out = np.zeros((B, t, C), np.float32)
    for c in range(N_CORES):
        b = c // (N_CORES // B)
        out[b] += results[c]["out"].T
    # bv folded out of the kernel: y@Wo.T picks up Wo @ bv per token
    out += (bo + Wo @ bv_full)[None, None, :]
    return out


def kernel(x, Wq, bq, Wk, bk, Wv, bv, Wo, bo, _trace=False, _tmpdir=None):
    x = np.asarray(x, dtype=np.float32)
    args = [np.asarray(a, dtype=np.float32) for a in (Wq, bq, Wk, bk, Wv, bv, Wo, bo)]
    Wq, bq, Wk, bk, Wv, bv, Wo, bo = args
    t = x.shape[1]
    in_maps = shard_inputs(x, Wq, bq, Wk, bk, Wv, bv, Wo, bo, t=t)
    res = run_cores(in_maps, t=t, trace=_trace, tmpdir=_tmpdir)
    out = gather(res.results, x.shape, Wv, bv, Wo, bo)
    kernel.last_result = res
    return out
